# revision 18
# baseline (speedup 1.0000x reference)
"""Cross-attention Trainium2 kernel (Bass/Tile), 8-core SPMD.

Problem: B=2, Tq=Tk=2048, C=1024, H=16 heads, D=64.
  q = query @ Wq + bq ; k,v = context @ Wkv + bkv (split)
  out = softmax(q k^T / sqrt(D)) v  @ Wo + bo

Sharding (data-parallel B x tensor-parallel heads):
  core c in 0..7 handles batch b = c//4 and head group hg = c%4
  (4 consecutive heads = 256 channels). Each core computes the partial
  out-projection  O_local @ Wo[rows of its heads]  (+ bo/4) and the host
  sums the 4 partials per batch (row-parallel Wo reduction).

Engine budget per core (the design targets):
  - ScalarE exp: 4 heads x 2048 x 2048 logits = 16.8M elements at
    1 elem/cycle/lane @1.2GHz -> ~110us floor. Batched as [128, 2, 512]
    (a head-pair's chunk) to amortize the ~210-cycle per-call overhead.
  - PE: scores have K=64 (head dim) so two heads are row-tiled into the
    128-row array concurrently (tile_position via base partitions 0/64,
    ~2x on the score matmuls). All matmul operands are bf16 (host-cast;
    fp8e4m3 DoubleRow projections were tried and fail the accuracy gate:
    e4m3's 2^-4 eps puts ~6e-2 on the output vs the 2e-2 budget).
  - Attention is swept per (head-pair p, 512-col q-block J) over 16
    key chunks of 128; PSUM: scores [128,2,512] f32 x2 bufs (8KB/part)
    + PV accumulator [65,2,512] (4KB) + projection pool 2x[128,512]
    (4KB) = 16KB/partition exactly.
  - The PV matmul accumulates a ones-column per head so the softmax
    denominator lands in PSUM row 64 for free; normalization is
    reciprocal (DVE) + partition_broadcast (GpSimd) + multiply (DVE).
  - Projections and the out-projection are woven into the attention
    chunk stream so the in-order PE executes them inside the exp-paced
    slack instead of serializing phases.
"""

import numpy as np

import concourse.bass as bass
import concourse.mybir as mybir
import concourse.tile as tile
from concourse import bacc
from concourse.bass_utils import run_bass_kernel_spmd

F32 = mybir.dt.float32
BF16 = mybir.dt.bfloat16
F8 = mybir.dt.float8e4
AF = mybir.ActivationFunctionType
DR = mybir.MatmulPerfMode.DoubleRow

T = 2048      # Tq = Tk
C = 1024      # embed dim
D = 64        # head dim
HL = 4        # heads per core
KT = C // 128  # 8 contraction tiles of 128 (4 DoubleRow tiles of 256)
NC_ = 16      # key chunks of 128
NJ = 4        # q blocks of 512
WS = 1.0      # no weight pre-scale needed at bf16
SCALE_EXP = float(D) ** -0.5 / (WS * WS)

_PROGRAMS = {}


def _emit(tc, sim_rowtile=False):
    nc = tc.nc
    qT = nc.dram_tensor("qT", [C, T], BF16, kind="ExternalInput").ap()
    cT = nc.dram_tensor("cT", [C, T], BF16, kind="ExternalInput").ap()
    wq = nc.dram_tensor("wq", [C, 256], BF16, kind="ExternalInput").ap()
    wk = nc.dram_tensor("wk", [C, 256], BF16, kind="ExternalInput").ap()
    wv = nc.dram_tensor("wv", [C, 256], BF16, kind="ExternalInput").ap()
    wo = nc.dram_tensor("wo", [256, C], BF16, kind="ExternalInput").ap()
    bq = nc.dram_tensor("bq", [256], F32, kind="ExternalInput").ap()
    bk = nc.dram_tensor("bk", [256], F32, kind="ExternalInput").ap()
    bv = nc.dram_tensor("bv", [256], F32, kind="ExternalInput").ap()
    bo4 = nc.dram_tensor("bo4", [C], F32, kind="ExternalInput").ap()
    out = nc.dram_tensor("out", [T, C], F32, kind="ExternalOutput").ap()

    from contextlib import ExitStack

    with ExitStack() as ctx:
        consts = ctx.enter_context(tc.tile_pool(name="consts", bufs=1))
        acts = ctx.enter_context(tc.tile_pool(name="acts", bufs=1))

        # DMA dispatch costs ~650ns serial time on the dispatching engine's
        # sequencer: few large DMAs, loads on SP, stores + small biases on
        # the near-idle GpSimd sequencer.
        # Prologue loads split across the two DMA queues so the two critical
        # chains (cT0+wk -> k-proj, qT0+wq -> q-proj) run in parallel.
        bk_sb = consts.tile([128, 2], F32, tag="bk")
        nc.gpsimd.dma_start(out=bk_sb, in_=bk.rearrange("(x p) -> p x", p=128))
        wk_sb = consts.tile([128, KT, 256], BF16, tag="wk")
        nc.sync.dma_start(out=wk_sb, in_=wk.rearrange("(t p) m -> p t m", p=128))
        wv_sb = consts.tile([128, KT, 256], BF16, tag="wv")

        def _pbcast(ap):
            return bass.AP(
                tensor=ap.tensor, offset=ap.offset, ap=[[0, 128]] + list(ap.ap)
            )

        bv_bc = consts.tile([128, 256], F32, tag="bv")
        # Warm the Exp activation table off the critical path (the first use
        # otherwise pays the ~2.7us table load at the start of attention).
        warm = consts.tile([1, 1], F32, tag="warm")
        nc.vector.memset(warm, 0.0)
        nc.scalar.activation(warm, warm, AF.Exp)

        # persistent projected activations (bf16)
        qt = [acts.tile([128, T], BF16, tag=f"qt{p}", name=f"qt{p}") for p in range(2)]
        kt = [acts.tile([128, T], BF16, tag=f"kt{p}", name=f"kt{p}") for p in range(2)]
        vt = [acts.tile([128, HL, D + 1], BF16, tag=f"v{i}", name=f"v{i}")
              for i in range(NC_)]
        ot = [acts.tile([128, T], BF16, tag=f"ot{p}", name=f"ot{p}") for p in range(2)]

        # All SBUF pools live for the whole program: recycling SBUF across
        # phase boundaries makes the first next-phase instruction on each
        # engine inherit WAW waits on all 8 DMA queues, which overflows the
        # ISA sync-wait table (walrus "Too many sync wait commands").
        ins_pool = ctx.enter_context(tc.tile_pool(name="ins", bufs=3))
        epool = ctx.enter_context(tc.tile_pool(name="att", bufs=3))
        smo = ctx.enter_context(tc.tile_pool(name="smo", bufs=2))
        sm1 = ctx.enter_context(tc.tile_pool(name="sm1", bufs=2))
        outs_pool = ctx.enter_context(tc.tile_pool(name="outs", bufs=3))

        qT_r = qT.rearrange("(t p) n -> p t n", p=128)
        cT_r = cT.rearrange("(t p) n -> p t n", p=128)

        wq_sb = consts.tile([128, KT, 256], BF16, tag="wq")
        wo_sb = consts.tile([128, 2, C], BF16, tag="wo")

        # PSUM pools (16KB/partition budget, exactly filled):
        ps_att = ctx.enter_context(tc.tile_pool(name="ps_att", bufs=2, space="PSUM"))
        ps_ov = ctx.enter_context(tc.tile_pool(name="ps_ov", bufs=1, space="PSUM"))
        pj = ctx.enter_context(tc.tile_pool(name="pj", bufs=2, space="PSUM"))

        cins = {}
        qins = {}

        def dma_ctx(j):
            cin = ins_pool.tile([128, KT, 512], BF16, tag="stage", name=f"cin{j}")
            nc.sync.dma_start(out=cin, in_=cT_r[:, :, j * 512:(j + 1) * 512])
            cins[j] = cin

        def dma_q(j):
            qin = ins_pool.tile([128, KT, 512], BF16, tag="stage", name=f"qin{j}")
            nc.sync.dma_start(out=qin, in_=qT_r[:, :, j * 512:(j + 1) * 512])
            qins[j] = qin

        def emit_ctx_k(j, ps_=None):
            sl = slice(j * 512, (j + 1) * 512)
            cin = cins[j]
            pairs = range(2) if ps_ is None else [ps_]
            for p in pairs:
                ps = pj.tile([128, 512], F32, tag="proj")
                for t in range(KT):
                    nc.tensor.matmul(
                        ps,
                        lhsT=wk_sb[:, t, p * 128:(p + 1) * 128],
                        rhs=cin[:, t, :],
                        start=(t == 0),
                        stop=(t == KT - 1),
                    )
                nc.vector.tensor_scalar_add(kt[p][:, sl], ps, bk_sb[:, p:p + 1])

        def emit_ctx_v(j, s):
            i = j * 4 + s
            cin = cins[j]
            pv = pj.tile([128, 512], F32, tag="proj")
            for t in range(KT):
                nc.tensor.matmul(
                    pv[:, 0:256],
                    lhsT=cin[:, t, s * 128:(s + 1) * 128],
                    rhs=wv_sb[:, t, :],
                    start=(t == 0),
                    stop=(t == KT - 1),
                )
            nc.vector.memset(vt[i][:, :, D:D + 1], 1.0)
            nc.vector.tensor_add(
                vt[i][:, :, 0:D],
                pv[:, 0:256].rearrange("p (h d) -> p h d", h=HL),
                bv_bc.rearrange("p (h d) -> p h d", h=HL),
            )

        def emit_q(j, p):
            sl = slice(j * 512, (j + 1) * 512)
            qin = qins[j]
            ps = pj.tile([128, 512], F32, tag="proj")
            for t in range(KT):
                nc.tensor.matmul(
                    ps,
                    lhsT=wq_sb[:, t, p * 128:(p + 1) * 128],
                    rhs=qin[:, t, :],
                    start=(t == 0),
                    stop=(t == KT - 1),
                )
            nc.vector.tensor_scalar_add(qt[p][:, sl], ps, bq_sb[:, p:p + 1])

        def emit_sweep(p, J, weave=None):
            """Attention for head pair p over q block J: 16 key chunks.

            Scores are emitted one chunk ahead of exp+PV (software pipeline
            depth 2) so ScalarE always has a scores tile ready and the PE's
            in-order queue never leaves it starved across sweep boundaries.
            """
            Jsl = slice(J * 512, (J + 1) * 512)
            ov = ps_ov.tile([65, 2, 512], F32, tag="ov")
            s_tiles = {}

            def emit_scores(i):
                s = ps_att.tile([128, 2, 512], F32, tag="s")
                for h in range(2):
                    hb = h * 64
                    # sim_rowtile shrinks the second (concurrent row-group)
                    # matmul so TimelineSim's serial PE model approximates
                    # the hardware's row-tiled concurrency.
                    N = 8 if (sim_rowtile and h == 1) else 512
                    nc.tensor.matmul(
                        s[:, h, 0:N],
                        lhsT=kt[p][hb:hb + 64, i * 128:(i + 1) * 128],
                        rhs=qt[p][hb:hb + 64, J * 512:J * 512 + N],
                        start=True,
                        stop=True,
                    )
                s_tiles[i] = s

            emit_scores(0)
            for i in range(NC_):
                if weave and i in weave:
                    for fn in weave[i]:
                        fn()
                if i + 1 < NC_:
                    emit_scores(i + 1)
                e = epool.tile([128, 2, 512], BF16, tag="e")
                nc.scalar.activation(e, s_tiles.pop(i), AF.Exp, scale=SCALE_EXP)
                for h in range(2):
                    nc.tensor.matmul(
                        ov[:, h, :],
                        lhsT=vt[i][:, 2 * p + h, :],
                        rhs=e[:, h, :],
                        start=(i == 0),
                        stop=(i == NC_ - 1),
                    )
            return ov

        def emit_norm(p, J, ov, c0, c1):
            # Normalize columns [c0:c1): reciprocal of the denominator row
            # straight out of PSUM first (so the GpSimd broadcast starts
            # early), evict rows 0..63 to SBUF to free the accumulator bank,
            # then multiply into ot.
            w = c1 - c0
            rec = sm1.tile([1, 2, w], F32, tag="rec")
            with nc.allow_low_precision(reason="f32 reciprocal"):
                nc.vector.reciprocal(rec, ov[64:65, :, c0:c1])
            osb = smo.tile([64, 2, w], F32, tag="osb")
            nc.vector.tensor_copy(osb, ov[0:64, :, c0:c1])
            bcast = sm1.tile([64, 2, w], F32, tag="bcast")
            nc.gpsimd.partition_broadcast(bcast, rec)
            for h in range(2):
                nc.vector.tensor_mul(
                    ot[p][h * 64:(h + 1) * 64, J * 512 + c0:J * 512 + c1],
                    osb[:, h, :], bcast[:, h, :]
                )

        def emit_sweep_normed(p, J, weave=None):
            ov = emit_sweep(p, J, weave)
            emit_norm(p, J, ov, 0, 512)

        def emit_outproj(Jb, qi, pool=None):
            qsl = slice(Jb * 512 + qi * 128, Jb * 512 + (qi + 1) * 128)
            ob = outs_pool.tile([128, 1024], F32, tag="ob")
            for ncol in range(2):
                csl = slice(ncol * 512, (ncol + 1) * 512)
                if pool is None:
                    po = pj.tile([128, 512], F32, tag="proj")
                else:
                    # tail only: rotate through the dead scores buffers
                    po = pool.tile([128, 512], F32, tag="s")
                nc.tensor.matmul(
                    po, lhsT=ot[0][:, qsl], rhs=wo_sb[:, 0, csl],
                    start=True, stop=False,
                )
                nc.tensor.matmul(
                    po, lhsT=ot[1][:, qsl], rhs=wo_sb[:, 1, csl],
                    start=False, stop=True,
                )
                nc.vector.tensor_add(ob[:, csl], po, bo_bc[:, csl])
            # Alternate store queues so the GpSimd sequencer (which also
            # runs the normalization broadcasts) never backs up.
            eng = nc.gpsimd if qi % 2 == 0 else nc.sync
            eng.dma_start(out=out[qsl, :], in_=ob)

        # ---- emission order = dataflow order, with weaving ----
        # Prologue: just enough to start the first sweep (kt/vt chunk 0 +
        # qt[0] block 0); everything else is woven into the exp-paced
        # chunk stream.
        dma_ctx(0)
        bq_sb = consts.tile([128, 2], F32, tag="bq")
        nc.gpsimd.dma_start(out=bq_sb, in_=bq.rearrange("(x p) -> p x", p=128))
        dma_q(0)
        nc.sync.dma_start(out=wq_sb, in_=wq.rearrange("(t p) m -> p t m", p=128))
        nc.sync.dma_start(out=wv_sb, in_=wv.rearrange("(t p) m -> p t m", p=128))
        nc.gpsimd.dma_start(out=bv_bc, in_=_pbcast(bv))
        emit_ctx_k(0)
        emit_q(0, 0)
        emit_ctx_v(0, 0)

        def dma_wo():
            nc.sync.dma_start(out=wo_sb, in_=wo.rearrange("(t p) m -> p t m", p=128))
            nc.gpsimd.dma_start(out=bo_bc, in_=_pbcast(bo4))

        bo_bc = consts.tile([128, C], F32, tag="bo")

        emit_sweep_normed(0, 0, weave={
            0: [lambda: dma_ctx(1), lambda: emit_ctx_v(0, 1)],
            1: [lambda: emit_ctx_v(0, 2)],
            2: [lambda: emit_ctx_v(0, 3)],
            3: [lambda: emit_ctx_k(1, 0)],
            4: [lambda: dma_ctx(2), lambda: emit_ctx_v(1, 0)],
            5: [lambda: emit_ctx_v(1, 1), lambda: emit_q(0, 1)],
            6: [lambda: emit_ctx_v(1, 2)],
            7: [lambda: emit_ctx_v(1, 3), lambda: emit_ctx_k(2, 0)],
            8: [lambda: dma_ctx(3), lambda: emit_ctx_v(2, 0)],
            9: [lambda: emit_ctx_v(2, 1)],
            10: [lambda: emit_ctx_v(2, 2)],
            11: [lambda: emit_ctx_v(2, 3), lambda: emit_ctx_k(3, 0)],
            12: [lambda: dma_q(1), lambda: emit_ctx_v(3, 0)],
            13: [lambda: emit_ctx_v(3, 1)],
            14: [lambda: emit_ctx_v(3, 2)],
            15: [lambda: emit_ctx_v(3, 3)],
        })
        emit_sweep_normed(1, 0, weave={
            0: [lambda: dma_q(2)],
            1: [lambda: emit_ctx_k(1, 1)],
            4: [lambda: emit_q(1, 0)],
            5: [lambda: emit_ctx_k(2, 1)],
            6: [lambda: emit_q(1, 1)],
            9: [lambda: emit_ctx_k(3, 1)],
            11: [dma_wo],
        })
        emit_sweep_normed(0, 1, weave={
            0: [lambda: emit_q(2, 0)],
            2: [lambda: emit_q(2, 1)],
            4: [lambda: emit_outproj(0, 0)],
            6: [lambda: emit_outproj(0, 1)],
            8: [lambda: emit_outproj(0, 2)],
            10: [lambda: emit_outproj(0, 3)],
            12: [lambda: dma_q(3)],
        })
        emit_sweep_normed(1, 1, weave={
            0: [lambda: emit_q(3, 0)],
            2: [lambda: emit_q(3, 1)],
        })
        emit_sweep_normed(0, 2, weave={
            4: [lambda: emit_outproj(1, 0)],
            6: [lambda: emit_outproj(1, 1)],
            8: [lambda: emit_outproj(1, 2)],
            10: [lambda: emit_outproj(1, 3)],
        })
        emit_sweep_normed(1, 2)
        emit_sweep_normed(0, 3, weave={
            4: [lambda: emit_outproj(2, 0)],
            6: [lambda: emit_outproj(2, 1)],
            8: [lambda: emit_outproj(2, 2)],
            10: [lambda: emit_outproj(2, 3)],
        })
        ov_f = emit_sweep(1, 3)
        # Fine-grained tail: normalize the final sweep per 256-column half
        # and start the dependent out-projection pieces immediately. The
        # attention PSUM pool is dead after the last exp, so alternate the
        # out-projection accumulators across both pools to run the PE four
        # tiles ahead of the bias-add stream.
        emit_norm(1, 3, ov_f, 0, 256)
        emit_outproj(3, 0, pool=ps_att)
        emit_norm(1, 3, ov_f, 256, 512)
        emit_outproj(3, 1)
        emit_outproj(3, 2, pool=ps_att)
        emit_outproj(3, 3)


def build_program(sim_rowtile=False):
    if sim_rowtile not in _PROGRAMS:
        nc = bacc.Bacc(
            "TRN2", target_bir_lowering=False, debug=False, num_devices=8
        )
        with tile.TileContext(nc) as tc:
            _emit(tc, sim_rowtile=sim_rowtile)
        # Bacc.compile() legalizes to the TRN2 1-wait-per-instruction
        # constraint (generate_event_semaphores) among other passes.
        nc.compile()
        _PROGRAMS[sim_rowtile] = nc
    return _PROGRAMS[sim_rowtile]


def make_in_maps(query, context, Wq, bq, Wkv, bkv, Wo, bo):
    BFNP = mybir.dt.np(BF16)
    query = np.asarray(query, dtype=np.float32)
    context = np.asarray(context, dtype=np.float32)
    Wq = np.asarray(Wq, dtype=np.float32)
    bq = np.asarray(bq, dtype=np.float32)
    Wkv = np.asarray(Wkv, dtype=np.float32)
    bkv = np.asarray(bkv, dtype=np.float32)
    Wo = np.asarray(Wo, dtype=np.float32)
    bo = np.asarray(bo, dtype=np.float32)

    qTs = [np.ascontiguousarray(query[b].T).astype(BFNP) for b in range(2)]
    cTs = [np.ascontiguousarray(context[b].T).astype(BFNP) for b in range(2)]
    in_maps = []
    for c in range(8):
        b, hg = c // 4, c % 4
        cs = slice(hg * 256, (hg + 1) * 256)
        vs = slice(1024 + hg * 256, 1024 + (hg + 1) * 256)
        in_maps.append(
            {
                "qT": qTs[b],
                "cT": cTs[b],
                "wq": np.ascontiguousarray(Wq[:, cs] * WS).astype(BFNP),
                "wk": np.ascontiguousarray(Wkv[:, cs] * WS).astype(BFNP),
                "wv": np.ascontiguousarray(Wkv[:, vs] * WS).astype(BFNP),
                "wo": np.ascontiguousarray(Wo[cs, :] / WS).astype(BFNP),
                "bq": np.ascontiguousarray(bq[cs] * WS),
                "bk": np.ascontiguousarray(bkv[cs] * WS),
                "bv": np.ascontiguousarray(bkv[vs] * WS),
                "bo4": np.ascontiguousarray(bo * 0.25),
            }
        )
    return in_maps


def combine(parts):
    """parts: list of 8 [T, C] partials -> [2, T, C] full output."""
    out = np.empty((2, T, C), dtype=np.float32)
    for b in range(2):
        acc = parts[4 * b].astype(np.float32, copy=True)
        for c in range(4 * b + 1, 4 * b + 4):
            acc += parts[c]
        out[b] = acc
    return out


def kernel(**inputs):
    nc = build_program()
    in_maps = make_in_maps(**inputs)
    res = run_bass_kernel_spmd(nc, in_maps, list(range(8)))
    parts = [res.results[c]["out"] for c in range(8)]
    return combine(parts)
